# revision 9
# baseline (speedup 1.0000x reference)
"""CPGNN (compatibility-guided GNN) kernel for 8 Trainium2 NeuronCores.

Reference computation (N=10000, F=512, HID=256, C=16, 4 post iterations):
    h      = relu(normed_adj @ (features @ W1) + b1)
    logits = normed_adj @ (h @ W2) + b2
    E_hat  = softmax(logits) - 1/C
    B_hat  = E_hat;  4x: B_hat = E_hat + raw_adj @ (B_hat @ H)
    out    = B_hat + 1/C

Sharding: rows of both adjacency matrices are sharded over the 8 cores
(1280 rows per core, tail core padded).  Adjacency shards are uploaded
TRANSPOSED and PAIR-MAJOR in fp8-e4m3 (scaled by 2^19 / 2^14) so the
on-device matmuls stream half the bytes of bf16.

Pipeline: phase 1 (X@W1) is fused tile-by-tile into phase 2's
accumulation loop; raw_adj is prefetched into SBUF during phases 1-4 so
all 4 post iterations run from SBUF.  hw2 and the per-iteration Y=B@H
matrices are all-gathered in 3 column CHUNKS each, pipelined against
the PE: each post iteration runs 3 chunk-major passes over the 80
k-tiles (k ordered by source chunk), completing output chunk c early so
its epilogue + gather overlap the remaining passes.  The E_hat term is
injected into PSUM via a tiny identity matmul (moving = bf16 SE*E_hat),
so the per-chunk epilogue is one scalar copy + one DVE add that emits
the next yb (= SE*B, bf16) directly.  H is pre-divided by SE host-side.

Measured: rel err ~4e-3 end to end.
"""

import numpy as np
import ml_dtypes

RANKS = 8
P = 128
NREAL = 10000
NK = 10240            # padded global row count
ML = 1280             # local rows per core
KT = NK // P          # 80 k-tiles
NPAIR = KT // 2       # 40 k-tile pairs
MT = ML // P          # 10
F = 512
FT = F // P           # 4
HID = 256
C = 16
NPOST = 4
NCACHE = 17           # adjn pairs cached in SBUF for phase 4 reuse
CHUNKS = [(0, 512), (512, 1024), (1024, 1280)]
# iteration/gather chunking: 2 chunks (768/512) — big chunk first so its
# gather launches early; pairs never split across chunks
CH2 = [(0, 768), (768, 1280)]
CH_MT = [(0, 6), (6, 10)]           # m-tile ranges per chunk
# sub-ranges per chunk: <=512 cols AND never crossing a 512-col (2KB)
# PSUM bank boundary — matmul accumulation is per-bank
SUBS = [[(0, 512), (512, 768)], [(768, 1024), (1024, 1280)]]

# NOTE: device float8e4 is e4m3 WITH inf/NaN (max finite 240, bytes
# >= 0x78 decode as inf/nan on the PE) — keep every fp8 value <= 224.
SA_N = 2.0 ** 19      # normed_adj fp8 scale (max ~107)
SA_R = 2.0 ** 14      # raw_adj fp8 scale (max ~164)
S_HW2 = 2.0 ** 12     # h@W2 fp8 scale (max ~130)
SE = SA_R             # B_hat carried as SE * B (bf16) on device

_CACHE = {}

# source-chunk-ordered k tiles: all (rank, m) with m in chunk 0, then
# chunk 1, then chunk 2 — matches the order gather chunks arrive.
KS_ORDER = [10 * r + m
            for c in range(len(CH_MT))
            for r in range(RANKS)
            for m in range(CH_MT[c][0], CH_MT[c][1])]


def _build_and_compile():
    import concourse.mybir as mybir
    import concourse.tile as tile
    from concourse import bacc

    dt = mybir.dt
    f32 = dt.float32
    bf16 = dt.bfloat16
    f8 = dt.float8e4
    AF = mybir.ActivationFunctionType

    nc = bacc.Bacc("TRN2", target_bir_lowering=False, debug=False,
                   num_devices=RANKS)

    adjn8 = nc.dram_tensor("adjn8", [NPAIR, P, 2 * ML], f8,
                           kind="ExternalInput").ap()
    adjr8 = nc.dram_tensor("adjr8", [NPAIR, P, 2 * ML], f8,
                           kind="ExternalInput").ap()
    xk = nc.dram_tensor("xk", [KT, P, F], bf16, kind="ExternalInput").ap()
    w1 = nc.dram_tensor("w1", [F, HID], bf16, kind="ExternalInput").ap()
    w2 = nc.dram_tensor("w2", [HID, C], bf16, kind="ExternalInput").ap()
    hmp = nc.dram_tensor("hmp", [C, C], bf16, kind="ExternalInput").ap()
    i16 = nc.dram_tensor("i16", [C, C], bf16, kind="ExternalInput").ap()
    b1 = nc.dram_tensor("b1", [HID, 1], f32, kind="ExternalInput").ap()
    b2c = nc.dram_tensor("b2c", [C, 1], f32, kind="ExternalInput").ap()
    outT = nc.dram_tensor("outT", [C, ML], f32, kind="ExternalOutput").ap()

    rg = [list(range(RANKS))]

    with tile.TileContext(nc) as tc:
        with tc.tile_pool(name="const", bufs=1) as const_pool, \
             tc.tile_pool(name="persist", bufs=1) as persist, \
             tc.tile_pool(name="res", bufs=1) as res_pool, \
             tc.tile_pool(name="dram", bufs=1, space="DRAM") as dram_pool:

            # ---- constants ----
            w1_sb = const_pool.tile([P, FT, HID], bf16)
            nc.sync.dma_start(w1_sb[:], w1.rearrange("(kt p) h -> p kt h", p=P))
            w2_sb = const_pool.tile([P, 2, C], bf16)
            nc.sync.dma_start(w2_sb[:], w2.rearrange("(kt p) c -> p kt c", p=P))
            hp_sb = const_pool.tile([C, C], bf16)
            nc.sync.dma_start(hp_sb[:], hmp[:])
            i16_sb = const_pool.tile([C, C], bf16)
            nc.sync.dma_start(i16_sb[:], i16[:])
            b1_sb = const_pool.tile([P, 2, 1], f32)
            nc.sync.dma_start(b1_sb[:], b1.rearrange("(t p) o -> p t o", p=P))
            b2c_sb = const_pool.tile([C, 1], f32)
            nc.sync.dma_start(b2c_sb[:], b2c[:])
            ones16_sb = const_pool.tile([C, 1], f32)
            nc.gpsimd.memset(ones16_sb[:], 1.0)
            ones1_sb = const_pool.tile([1, C], f32)
            nc.gpsimd.memset(ones1_sb[:], 1.0)

            # ---- warm-up collectives (overlap phase 1/2 compute) ----
            wu_in = dram_pool.tile([C, 1], f32)
            nc.sync.dma_start(wu_in[:], ones16_sb[:])
            for wi in range(3):
                wu_out = dram_pool.tile([RANKS * C, 1], f32,
                                        name=f"wu_out{wi}",
                                        addr_space="Shared")
                nc.gpsimd.collective_compute(
                    "AllGather", mybir.AluOpType.bypass, replica_groups=rg,
                    ins=[wu_in[:].opt()], outs=[wu_out[:].opt()])
            # dtype/size-matched warmups for the real hw2 (fp8) and Y (bf16)
            # gathers so their first use is not a cold ring
            wu8_sb = const_pool.tile([P, 96], f8)
            nc.gpsimd.memset(wu8_sb[:], 0.0)
            wu8_in = dram_pool.tile([P * 96], f8, name="wu8_in")
            nc.sync.dma_start(wu8_in.rearrange("(p a) -> p a", p=P),
                              wu8_sb[:])
            wu8_out = dram_pool.tile([RANKS * P * 96], f8, name="wu8_out",
                                     addr_space="Shared")
            nc.gpsimd.collective_compute(
                "AllGather", mybir.AluOpType.bypass, replica_groups=rg,
                ins=[wu8_in[:].opt()], outs=[wu8_out[:].opt()])
            wub_sb = const_pool.tile([P, 96], bf16)
            nc.gpsimd.memset(wub_sb[:], 0.0)
            wub_in = dram_pool.tile([P * 96], bf16, name="wub_in")
            nc.sync.dma_start(wub_in.rearrange("(p a) -> p a", p=P),
                              wub_sb[:])
            wub_out = dram_pool.tile([RANKS * P * 96], bf16, name="wub_out",
                                     addr_space="Shared")
            nc.gpsimd.collective_compute(
                "AllGather", mybir.AluOpType.bypass, replica_groups=rg,
                ins=[wub_in[:].opt()], outs=[wub_out[:].opt()])

            # ---- persistent intermediates ----
            h1t_sb = persist.tile([P, 2, ML], bf16)        # h.T  [HID, ML]
            # gathered h@W2 / Y: one tile per source chunk (contiguous DMA)
            hw2g = [persist.tile([P, RANKS, (mhi - mlo) * C], f8,
                                 name=f"hw2g{ci}")
                    for ci, (mlo, mhi) in enumerate(CH_MT)]
            y_g = [[persist.tile([P, RANKS, (mhi - mlo) * C], bf16,
                                 name=f"y_g{i}_{ci}")
                    for ci, (mlo, mhi) in enumerate(CH_MT)]
                   for i in range(2)]                      # double-buffered
            e15b_sb = persist.tile([C, ML], bf16)          # SE * E_hat.T (bf16)
            ybt_sb = persist.tile([C, ML], bf16)           # SE * B.T per iter
            adjr_res = res_pool.tile([P, NPAIR, 2, ML], f8)  # full raw shard

            # ====== fused phase 1+2: XW1 tiles -> h.T accumulation =========
            with tc.tile_pool(name="adjnc", bufs=1) as adjnc:
                adjn_res = adjnc.tile([P, NCACHE, 2, ML], f8)
                with tc.tile_pool(name="xs", bufs=6) as xs, \
                     tc.tile_pool(name="xw1p", bufs=4) as xw1p, \
                     tc.tile_pool(name="ph2s", bufs=4) as ph2s, \
                     tc.tile_pool(name="ps1", bufs=2, space="PSUM") as ps1, \
                     tc.tile_pool(name="ps2", bufs=1, space="PSUM") as ps2:
                    psum_h0 = ps2.tile([P, ML], f32, name="psum_h0")
                    psum_h1 = ps2.tile([P, ML], f32, name="psum_h1")
                    psum_h = [psum_h0, psum_h1]

                    def ph1_tile(k):
                        xt = xs.tile([P, FT, P], bf16, name="xt")
                        nc.sync.dma_start(
                            xt[:], xk[k].rearrange("p (kf j) -> p kf j", kf=FT))
                        psum1 = ps1.tile([P, HID], f32, name="psum1")
                        for kf in range(FT):
                            nc.tensor.matmul(psum1[:], xt[:, kf, :],
                                             w1_sb[:, kf, :],
                                             start=(kf == 0),
                                             stop=(kf == FT - 1))
                        xw1k = xw1p.tile([P, HID], bf16, name="xw1k")
                        nc.scalar.activation(xw1k[:], psum1[:], AF.Copy)
                        return xw1k

                    def ph2_tile(k, xw1k, src):
                        # src: [P, ML] fp8 view of adjn k-tile
                        for mh in range(2):
                            for (n0, n1) in CHUNKS:
                                nc.tensor.matmul(
                                    psum_h[mh][:, n0:n1],
                                    xw1k[:, mh * P:(mh + 1) * P],
                                    src[:, n0:n1],
                                    start=(k == 0), stop=(k == KT - 1))

                    prev = None  # (k, xw1k, src)
                    for k in range(KT):
                        pr, half = divmod(k, 2)
                        if half == 0:
                            if pr < NCACHE:
                                pair_t = adjn_res[:, pr, :, :]
                            else:
                                pair_t = ph2s.tile([P, 2, ML], f8,
                                                   name="adjn_k")
                            nc.sync.dma_start(
                                pair_t[:],
                                adjn8[pr].rearrange("p (two m) -> p two m",
                                                    two=2))
                            # interleave raw-adj prefetch 1:1 with adjn stream
                            nc.sync.dma_start(
                                adjr_res[:, pr, :, :],
                                adjr8[pr].rearrange("p (two m) -> p two m",
                                                    two=2))
                            cur_pair = pair_t
                        xw1k = ph1_tile(k)
                        if k == 60:
                            # skew-absorbing sync: cores align here on the CC
                            # engine while the PE keeps processing k=61..79,
                            # so the phase-3 hw2 gather sees minimal skew
                            ws_in = dram_pool.tile([P, 96], bf16,
                                                   name="ws_in")
                            nc.sync.dma_start(ws_in[:], xw1k[:, 0:96])
                            ws_out = dram_pool.tile([RANKS * P, 96], bf16,
                                                    name="ws_out",
                                                    addr_space="Shared")
                            nc.gpsimd.collective_compute(
                                "AllGather", mybir.AluOpType.bypass,
                                replica_groups=rg,
                                ins=[ws_in[:].opt()],
                                outs=[ws_out[:].opt()])
                        if prev is not None:
                            ph2_tile(*prev)
                        prev = (k, xw1k, cur_pair[:, half, :])
                    ph2_tile(*prev)
                    for mh in range(2):
                        nc.scalar.activation(h1t_sb[:, mh, :], psum_h[mh][:],
                                             AF.Relu, bias=b1_sb[:, mh, :],
                                             scale=1.0 / SA_N)

                # ====== phase 3: hw2 = h @ W2 -> fp8, chunked all-gather ====
                with tc.tile_pool(name="ph3", bufs=1) as ph3, \
                     tc.tile_pool(name="ps3", bufs=4, space="PSUM") as ps3:
                    hw2_sb = ph3.tile([P, MT, C], f8)
                    for ci, (mlo, mhi) in enumerate(CH_MT):
                        for m in range(mlo, mhi):
                            psum3 = ps3.tile([P, C], f32, name="psum3")
                            for kh in range(2):
                                nc.tensor.matmul(
                                    psum3[:],
                                    h1t_sb[:, kh, m * P:(m + 1) * P],
                                    w2_sb[:, kh, :],
                                    start=(kh == 0), stop=(kh == 1))
                            nc.scalar.activation(hw2_sb[:, m, :], psum3[:],
                                                 AF.Copy, scale=S_HW2)
                        mlen = mhi - mlo
                        hw2c_dram = dram_pool.tile([P, mlen * C], f8,
                                                   name=f"hw2c{ci}")
                        nc.sync.dma_start(
                            hw2c_dram.rearrange("p (mt c) -> p mt c", mt=mlen),
                            hw2_sb[:, mlo:mhi, :])
                        hw2f_dram = dram_pool.tile([RANKS * P, mlen * C], f8,
                                                   name=f"hw2f{ci}",
                                                   addr_space="Shared")
                        nc.gpsimd.collective_compute(
                            "AllGather", mybir.AluOpType.bypass,
                            replica_groups=rg,
                            ins=[hw2c_dram[:].opt()],
                            outs=[hw2f_dram[:].opt()])
                        hw2f_v = hw2f_dram.rearrange("(r p) a -> p r a",
                                                     p=P)
                        nc.sync.dma_start(hw2g[ci][:, 0:4, :],
                                          hw2f_v[:, 0:4, :])
                        nc.sync.dma_start(hw2g[ci][:, 4:8, :],
                                          hw2f_v[:, 4:8, :])

                # ====== phase 4: logits via fp8 skinny-stationary ===========
                with tc.tile_pool(name="ph4s", bufs=6) as ph4s, \
                     tc.tile_pool(name="ph4", bufs=1) as ph4, \
                     tc.tile_pool(name="ps4", bufs=1, space="PSUM") as ps4:
                    psum_l = ps4.tile([P, ML], f32, name="psum_l",
                                      tag="ph4big")
                    stream_tiles = {}
                    for idx, k in enumerate(KS_ORDER):
                        pr, half = divmod(k, 2)
                        if pr < NCACHE:
                            src = adjn_res[:, pr, half, :]
                        else:
                            if half == 0:
                                st = ph4s.tile([P, 2, ML], f8, name="adjn_k2")
                                nc.sync.dma_start(
                                    st[:],
                                    adjn8[pr].rearrange(
                                        "p (two m) -> p two m", two=2))
                                stream_tiles[pr] = st
                            src = stream_tiles[pr][:, half, :]
                        j = idx % 2
                        kci = 0 if k % MT < CH_MT[0][1] else 1
                        mi = k % MT - CH_MT[kci][0]
                        stat = hw2g[kci][:, k // MT, mi * C:(mi + 1) * C]
                        for (n0, n1) in CHUNKS:
                            nc.tensor.matmul(
                                psum_l[32 * j:32 * j + C, n0:n1],
                                stat, src[:, n0:n1],
                                start=(idx < 2), stop=(idx >= KT - 2),
                                tile_position=(0, 32 * j))
                    # softmax tail chunked by CH2 so each chunk's e15b is
                    # ready (and its Y0 gather can launch) ASAP
                    l1_sb = ph4.tile([C, ML], f32)
                    lsum_sb = ph4.tile([C, ML], f32)
                    expT_sb = ph4.tile([C, ML], f32)
                    sums_ps = ps4.tile([1, ML], f32, name="sums_ps",
                                       tag="soft")
                    sumsr_sb = ph4.tile([1, ML], f32)
                    bc_ps = ps4.tile([C, ML], f32, name="bc_ps", tag="soft")
                    rcp_sb = ph4.tile([C, ML], f32)
                    prior_sb = ph4.tile([C, ML], f32)
                    softmax_done = []
                    for ci, (c0, c1) in enumerate(CH2):
                        nc.scalar.activation(l1_sb[:, c0:c1],
                                             psum_l[32:32 + C, c0:c1],
                                             AF.Copy)
                        nc.vector.tensor_add(lsum_sb[:, c0:c1],
                                             psum_l[0:C, c0:c1],
                                             l1_sb[:, c0:c1])
                        nc.scalar.activation(expT_sb[:, c0:c1],
                                             lsum_sb[:, c0:c1], AF.Exp,
                                             bias=b2c_sb[:],
                                             scale=1.0 / (SA_N * S_HW2))
                        for (n0, n1) in SUBS[ci]:
                            nc.tensor.matmul(sums_ps[:, n0:n1], ones16_sb[:],
                                             expT_sb[:, n0:n1],
                                             start=True, stop=True)
                        nc.scalar.activation(sumsr_sb[:, c0:c1],
                                             sums_ps[:, c0:c1], AF.Copy)
                        for (n0, n1) in SUBS[ci]:
                            nc.tensor.matmul(bc_ps[:, n0:n1], ones1_sb[:],
                                             sumsr_sb[:, n0:n1],
                                             start=True, stop=True)
                        nc.vector.reciprocal_approx_fast(rcp_sb[:, c0:c1],
                                                         bc_ps[:, c0:c1])
                        nc.vector.tensor_mul(prior_sb[:, c0:c1],
                                             expT_sb[:, c0:c1],
                                             rcp_sb[:, c0:c1])
                        nc.scalar.activation(e15b_sb[:, c0:c1],
                                             prior_sb[:, c0:c1], AF.Copy,
                                             scale=SE, bias=-SE / C)
                        softmax_done.append(ci)

            # ====== phase 5: post-process iterations ========================
            with tc.tile_pool(name="ph5", bufs=2) as ph5, \
                 tc.tile_pool(name="ps5y", bufs=2, space="PSUM") as ps5y, \
                 tc.tile_pool(name="ps5b", bufs=1, space="PSUM") as ps5b:
                psum_b = ps5b.tile([P, ML], f32, name="psum_b")

                def y_chunk_gather(yb, it, ci):
                    """Y[:, chunk ci] = (SE*B)@(H/SE) locally, gather chunk."""
                    mlo, mhi = CH_MT[ci]
                    mlen = mhi - mlo
                    psum_y = ps5y.tile([P, mlen, C], f32, name="psum_y")
                    for mi, m in enumerate(range(mlo, mhi)):
                        nc.tensor.matmul(psum_y[:, mi, :],
                                         yb[:, m * P:(m + 1) * P],
                                         hp_sb[:], start=True, stop=True)
                    yloc_sb = ph5.tile([P, mlen, C], bf16, name="yloc")
                    nc.scalar.activation(yloc_sb[:], psum_y[:], AF.Copy)
                    yloc_dram = dram_pool.tile([P, mlen * C], bf16,
                                               name=f"yloc{it}_{ci}")
                    nc.sync.dma_start(
                        yloc_dram.rearrange("p (mt c) -> p mt c", mt=mlen),
                        yloc_sb[:])
                    yfull = dram_pool.tile([RANKS * P, mlen * C], bf16,
                                           name=f"yfull{it}_{ci}",
                                           addr_space="Shared")
                    nc.gpsimd.collective_compute(
                        "AllGather", mybir.AluOpType.bypass, replica_groups=rg,
                        ins=[yloc_dram[:].opt()], outs=[yfull[:].opt()])
                    yfull_v = yfull.rearrange("(r p) a -> p r a", p=P)
                    nc.sync.dma_start(y_g[it % 2][ci][:, 0:4, :],
                                      yfull_v[:, 0:4, :])
                    nc.sync.dma_start(y_g[it % 2][ci][:, 4:8, :],
                                      yfull_v[:, 4:8, :])

                # Y0 gathers from e15b (SE*E_hat): Y0 = E@H
                for ci in range(len(CH_MT)):
                    y_chunk_gather(e15b_sb, 0, ci)

                # deferred epilogue events: emit after a few matmuls of the
                # next pass so the PE never waits on scalar/vector work
                pending = []

                def flush_pending():
                    for fn in pending:
                        fn()
                    pending.clear()

                for it in range(NPOST):
                    yg = y_g[it % 2]
                    for ci in range(len(CH2)):
                        # E-injection opens strip0's accumulation groups
                        for (n0, n1) in SUBS[ci]:
                            nc.tensor.matmul(
                                psum_b[0:C, n0:n1], i16_sb[:],
                                e15b_sb[:, n0:n1],
                                start=True, stop=False,
                                tile_position=(0, 0))
                        for idx, k in enumerate(KS_ORDER):
                            if idx == 6:
                                flush_pending()
                            j = idx % 2
                            kci = 0 if k % MT < CH_MT[0][1] else 1
                            mi = k % MT - CH_MT[kci][0]
                            stat = yg[kci][:, k // MT, mi * C:(mi + 1) * C]
                            for (n0, n1) in SUBS[ci]:
                                nc.tensor.matmul(
                                    psum_b[32 * j:32 * j + C, n0:n1],
                                    stat,
                                    adjr_res[:, k // 2, k % 2, n0:n1],
                                    start=(j == 1 and idx == 1),
                                    stop=(idx >= KT - 2),
                                    tile_position=(0, 32 * j))
                        flush_pending()

                        n0, n1 = CH2[ci]

                        def make_epilogue(it=it, ci=ci, n0=n0, n1=n1):
                            def ep():
                                t1 = ph5.tile([C, n1 - n0], f32, name="t1")
                                nc.scalar.activation(
                                    t1[:], psum_b[32:32 + C, n0:n1], AF.Copy)
                                if it < NPOST - 1:
                                    nc.vector.tensor_add(
                                        ybt_sb[:, n0:n1],
                                        psum_b[0:C, n0:n1], t1[:])
                                    y_chunk_gather(ybt_sb, it + 1, ci)
                                else:
                                    btf = ph5.tile([C, n1 - n0], f32,
                                                   name="btf")
                                    nc.vector.tensor_add(
                                        btf[:], psum_b[0:C, n0:n1], t1[:])
                                    outc = ph5.tile([C, n1 - n0], f32,
                                                    name="outc")
                                    nc.scalar.activation(
                                        outc[:], btf[:], AF.Copy,
                                        scale=1.0 / SE, bias=1.0 / C)
                                    nc.sync.dma_start(outT[:, n0:n1], outc[:])
                            return ep
                        pending.append(make_epilogue())
                flush_pending()

    nc.compile()
    return nc


def _get_compiled():
    if "nc" not in _CACHE:
        _CACHE["nc"] = _build_and_compile()
    return _CACHE["nc"]


def _pair_major_fp8(adj_shard_T, scale):
    """[NK, ML] f32 (transposed shard) -> pair-major fp8 [NPAIR, P, 2*ML]."""
    e4 = ml_dtypes.float8_e4m3fn
    a = np.clip(adj_shard_T * np.float32(scale), 0.0, 224.0)
    a = a.reshape(NPAIR, 2, P, ML).transpose(0, 2, 1, 3).reshape(
        NPAIR, P, 2 * ML)
    return np.ascontiguousarray(a).astype(e4)


def _prep_inputs(raw_adj, normed_adj, features, W1, b1, W2, b2, H):
    bf = ml_dtypes.bfloat16
    xpad = np.zeros((NK, F), dtype=np.float32)
    xpad[:NREAL] = features
    # xk[k, p, (kf j)] = X[k*128+j, kf*128+p]
    xkarr = np.ascontiguousarray(
        xpad.reshape(KT, P, FT, P).transpose(0, 3, 2, 1).reshape(KT, P, F)
    ).astype(bf)
    w1b = np.ascontiguousarray(W1).astype(bf)
    w2b = np.ascontiguousarray(W2).astype(bf)
    hpb = np.ascontiguousarray(np.asarray(H, dtype=np.float64) / SE).astype(bf)
    i16b = np.eye(C, dtype=np.float32).astype(bf)
    b1c = np.asarray(b1, dtype=np.float32).reshape(HID, 1).copy()
    b2col = np.asarray(b2, dtype=np.float32).reshape(C, 1).copy()
    in_maps = []
    for r in range(RANKS):
        r0 = r * ML
        r1 = min(r0 + ML, NREAL)
        nr = r1 - r0
        an = np.zeros((NK, ML), dtype=np.float32)
        an[:NREAL, :nr] = normed_adj[r0:r1].T
        ar = np.zeros((NK, ML), dtype=np.float32)
        ar[:NREAL, :nr] = raw_adj[r0:r1].T
        in_maps.append({
            "adjn8": _pair_major_fp8(an, SA_N),
            "adjr8": _pair_major_fp8(ar, SA_R),
            "xk": xkarr, "w1": w1b, "w2": w2b,
            "hmp": hpb, "i16": i16b, "b1": b1c, "b2c": b2col,
        })
    return in_maps


def run_on_device(in_maps, trace=False):
    from concourse import bass_utils
    nc = _get_compiled()
    return bass_utils.run_bass_kernel_spmd(
        nc, in_maps, core_ids=list(range(RANKS)), trace=trace)


def kernel(raw_adj, normed_adj, features, y_onehot, train_mask,
           W1, b1, W2, b2, H):
    in_maps = _prep_inputs(np.asarray(raw_adj), np.asarray(normed_adj),
                           np.asarray(features), np.asarray(W1),
                           np.asarray(b1), np.asarray(W2), np.asarray(b2),
                           np.asarray(H))
    res = run_on_device(in_maps)
    parts = []
    for r in range(RANKS):
        o = np.asarray(res.results[r]["outT"], dtype=np.float32)  # [C, ML]
        parts.append(o.T)
    full = np.concatenate(parts, axis=0)[:NREAL]
    return np.ascontiguousarray(full).astype(np.float32)
